# revision 43
# baseline (speedup 1.0000x reference)
"""Fused GroupNorm + self-attention + proj + residual block for TRN2, v3.

Data-parallel over batch (B=8 = 8 cores). Measured-rate-optimal:
- QK in bf16, regular [128]-contract matmuls (1 c/row, same as fp8 on HW).
- PV + denominator in fp8 DoubleRow over paired j-tiles ([128,2,*], 256-deep
  contraction = 2x effective rate). Denominator via ones-matmul on PE.
- exp split across Scalar (native exp -> fp8e5) and Vector (Schraudolph
  int8 bit-trick -> bitcast fp8e5). GPSIMD avoided (software-slow, no PSUM).
"""

import os
import sys
from contextlib import ExitStack

for _p in ("/opt/trn_rl_repo", "/opt/pypackages"):
    if _p not in sys.path:
        sys.path.append(_p)

import numpy as np

import concourse.bass as bass
import concourse.tile as tile
from concourse import mybir

C = 128
N = 4096
GROUPS = 8
GSIZE = C // GROUPS
EPS = 1e-5
NCORES = 8
CHUNK = 512
NCHUNK = N // CHUNK
JT = 128
NJT = N // JT
NPAIR = NJT // 2

F32 = mybir.dt.float32
BF16 = mybir.dt.bfloat16
F8E4 = mybir.dt.float8e4
F8E5 = mybir.dt.float8e5
I8 = mybir.dt.int8
AF = mybir.ActivationFunctionType
ALU = mybir.AluOpType
DR = mybir.MatmulPerfMode.DoubleRow

F_SCALE = C ** -0.5               # score scale, applied inside exp
EBIAS = -2.0                      # exp(s + EBIAS), cancels in normalization
SCH_A = 4.0 / np.log(2.0)         # e5m2 Schraudolph
SCH_B = 60.0 - 4.0 * 0.0430
# exp producer per pair: A=Scalar(ACT), D=Vector(DVE)
EXP_SCHED = "ADAADADAADADAADA"    # 10xA, 6xD


def attention_block_tile(tc, outs, ins):
    nc = tc.nc
    x_d = ins["x"]
    wqT_d = ins["wqT"]
    wkT_d = ins["wkT"]
    wvT_d = ins["wvT"]
    projT_d = ins["projT"]
    bq_d = ins["bq"]
    pbe_d = ins["pbe"]
    gnw_d = ins["gn_w"]
    gnb_d = ins["gn_b"]
    gind_d = ins["g_ind"]
    gbc_d = ins["g_bcast"]
    out_d = outs["out"]

    ctx = ExitStack()
    const = ctx.enter_context(tc.tile_pool(name="const", bufs=1))
    big = ctx.enter_context(tc.tile_pool(name="big", bufs=1))
    small = ctx.enter_context(tc.tile_pool(name="small", bufs=2))
    etile = ctx.enter_context(tc.tile_pool(name="etile", bufs=6))
    chunkp = ctx.enter_context(tc.tile_pool(name="chunkp", bufs=4))
    psum_s = ctx.enter_context(tc.tile_pool(name="psum_s", bufs=2, space="PSUM"))
    psum_o = ctx.enter_context(tc.tile_pool(name="psum_o", bufs=1, space="PSUM"))
    psum_d = ctx.enter_context(tc.tile_pool(name="psum_d", bufs=2, space="PSUM"))
    psum_p = ctx.enter_context(tc.tile_pool(name="psum_p", bufs=1, space="PSUM"))

    dma = nc.sync

    # ---- input DMA first: x chunks gate GroupNorm stats, so their
    # descriptors go to the head of both HWDGE queues ----
    x_sb = big.tile([C, N], F32, tag="x")
    for s in range(8):
        eng = nc.sync if s % 2 == 0 else nc.scalar
        eng.dma_start(out=x_sb[:, s * 512:(s + 1) * 512],
                      in_=x_d[:, s * 512:(s + 1) * 512])

    # ---- constants ----
    wqT_f = const.tile([C, C], F32, tag="wqT_f")
    wkT_f = const.tile([C, C], F32, tag="wkT_f")
    wvT_f = const.tile([C, C], F32, tag="wvT_f")
    projT_f = const.tile([C, C], F32, tag="projT_f")
    dma.dma_start(out=wqT_f, in_=wqT_d)
    dma.dma_start(out=wkT_f, in_=wkT_d)
    dma.dma_start(out=wvT_f, in_=wvT_d)
    dma.dma_start(out=projT_f, in_=projT_d)
    wqT = const.tile([C, C], BF16, tag="wqT")
    wkT = const.tile([C, C], BF16, tag="wkT")
    wvT = const.tile([C, C], BF16, tag="wvT")
    projT = const.tile([C, C], BF16, tag="projT")
    nc.vector.tensor_copy(out=wqT, in_=wqT_f)
    nc.vector.tensor_copy(out=wkT, in_=wkT_f)
    nc.vector.tensor_copy(out=wvT, in_=wvT_f)
    nc.vector.tensor_copy(out=projT, in_=projT_f)
    bq = const.tile([C, 1], F32, tag="bq")
    pbe = const.tile([C, 1], F32, tag="pbe")
    gnw = const.tile([C, 1], F32, tag="gnw")
    gnb = const.tile([C, 1], F32, tag="gnb")
    dma.dma_start(out=bq, in_=bq_d)
    dma.dma_start(out=pbe, in_=pbe_d)
    dma.dma_start(out=gnw, in_=gnw_d)
    dma.dma_start(out=gnb, in_=gnb_d)
    gind = const.tile([C, GROUPS], F32, tag="gind")
    gbc = const.tile([GROUPS, C], F32, tag="gbc")
    dma.dma_start(out=gind, in_=gind_d)
    dma.dma_start(out=gbc, in_=gbc_d)
    ones8 = const.tile([128, 2, 32], F8E4, tag="ones8")
    nc.vector.memset(ones8, 1.0)
    ones1 = const.tile([1, C], BF16, tag="ones1")
    nc.vector.memset(ones1, 1.0)
    ebias_t = const.tile([C, 1], F32, tag="ebias")
    nc.vector.memset(ebias_t, EBIAS)
    warm = const.tile([1, 1], F32, tag="warm")
    nc.vector.memset(warm, 1.0)
    nc.scalar.activation(out=warm, in_=warm, func=AF.Ln)

    # ---- HAM warmup: keep PE busy while the x DMA + GN stats run, so the
    # PE clock gate is already 8/8 when the real matmul burst starts ----
    wt = const.tile([128, 512], BF16, tag="warm_mm")
    nc.vector.memset(wt, 0.25)
    for wi in range(12):
        wp = psum_s.tile([C, CHUNK], F32, tag="s2", padded_shape=[C, 2 * CHUNK],
                         name=f"warm_{wi}")
        nc.tensor.matmul(wp, lhsT=wt[:, 0:128], rhs=wt, start=True, stop=True)

    # ---- GN stats ----
    stats = small.tile([C, 8, 6], F32, tag="gn_stats")
    for s in range(8):
        nc.vector.bn_stats(out=stats[:, s, :], in_=x_sb[:, s * 512:(s + 1) * 512])
    mv = small.tile([C, 2], F32, tag="gn_mv")
    nc.vector.bn_aggr(out=mv, in_=stats)
    stat2 = small.tile([C, 2], F32, tag="gn_stat2")
    nc.vector.tensor_copy(out=stat2[:, 0:1], in_=mv[:, 0:1])
    # E[x^2] = mean^2 + var in one fused op
    nc.vector.scalar_tensor_tensor(out=stat2[:, 1:2], in0=mv[:, 0:1],
                                   scalar=mv[:, 0:1], in1=mv[:, 1:2],
                                   op0=ALU.mult, op1=ALU.add)
    gstats_ps = psum_p.tile([GROUPS, 2], F32, tag="p")
    nc.tensor.matmul(gstats_ps, lhsT=gind, rhs=stat2, start=True, stop=True)
    gstats = small.tile([GROUPS, 2], F32, tag="gn_gstats")
    nc.vector.tensor_copy(out=gstats, in_=gstats_ps)
    # fused: nvar = mean^2 - E[x^2] = -var, then Ln(-1*nvar + eps)
    nvar = small.tile([GROUPS, 1], F32, tag="gn_nvar")
    nc.vector.scalar_tensor_tensor(out=nvar, in0=gstats[:, 0:1],
                                   scalar=gstats[:, 0:1], in1=gstats[:, 1:2],
                                   op0=ALU.mult, op1=ALU.subtract)
    eps_t = const.tile([GROUPS, 1], F32, tag="eps")
    nc.vector.memset(eps_t, EPS)
    glnv = small.tile([GROUPS, 1], F32, tag="gn_glnv")
    nc.scalar.activation(out=glnv, in_=nvar, func=AF.Ln, scale=-1.0, bias=eps_t)
    nc.scalar.activation(out=gstats[:, 1:2], in_=glnv, func=AF.Exp, scale=-0.5)
    chst_ps = psum_p.tile([C, 2], F32, tag="p")
    nc.tensor.matmul(chst_ps, lhsT=gbc, rhs=gstats, start=True, stop=True)
    # read broadcast stats straight from PSUM: saves the copy-back hop
    scale = small.tile([C, 1], F32, tag="gn_scale")
    nc.vector.tensor_mul(out=scale, in0=gnw, in1=chst_ps[:, 1:2])
    sm = small.tile([C, 1], F32, tag="gn_sm")
    nc.vector.tensor_mul(out=sm, in0=chst_ps[:, 0:1], in1=scale)
    shift = small.tile([C, 1], F32, tag="gn_shift")
    nc.vector.tensor_tensor(out=shift, in0=gnb, in1=sm, op=ALU.subtract)

    # ---- P1: h, q, k (bf16), vT (fp8e4) per chunk ----
    h_sb = big.tile([C, N], BF16, tag="h")
    q_sb = big.tile([C, N], BF16, tag="q")
    k_sb = big.tile([C, N], BF16, tag="k")
    vT = big.tile([128, NJT * C], F8E4, tag="vT")
    sch_a1 = SCH_A * F_SCALE
    sch_a2 = SCH_B + EBIAS * SCH_A
    pending_tail = [None]

    def run_tail():
        if pending_tail[0] is not None:
            pending_tail[0]()
            pending_tail[0] = None

    gpend = []

    def start_chunk(ic):
        return {
            "ic": ic,
            "isl": slice(ic * CHUNK, (ic + 1) * CHUNK),
            "o_ps": psum_o.tile([C, CHUNK], F32, tag="o", name=f"o_{ic}"),
            "sums_ps": psum_d.tile([32, CHUNK], F32, tag="d", name=f"sums_{ic}"),
            "etiles": {},
        }

    def flush(st, g):
        e2 = st["etiles"].pop(g).bitcast(F8E5).rearrange("p (t i) -> p t i", t=2)
        nc.tensor.matmul(st["o_ps"], lhsT=vT[:, 2 * g * C:(2 * g + 2) * C]
                         .rearrange("p (t c) -> p t c", t=2),
                         rhs=e2, start=(g == 0), stop=(g == NPAIR - 1),
                         perf_mode=DR)
        nc.tensor.matmul(st["sums_ps"], lhsT=ones8,
                         rhs=e2, start=(g == 0), stop=(g == NPAIR - 1),
                         perf_mode=DR, skip_group_check=True)

    def do_pair(st, g):
        isl = st["isl"]
        s2_ps = psum_s.tile([C, 2 * CHUNK], F32, tag="s2")
        for t in range(2):
            jt = 2 * g + t
            nc.tensor.matmul(s2_ps[:, t * CHUNK:(t + 1) * CHUNK],
                             lhsT=k_sb[:, jt * JT:(jt + 1) * JT],
                             rhs=q_sb[:, isl], start=True, stop=True)
        e = etile.tile([C, 2 * CHUNK], I8, tag="e")
        if EXP_SCHED[g] == "A":
            nc.scalar.activation(out=e.bitcast(F8E5), in_=s2_ps, func=AF.Exp,
                                 scale=F_SCALE, bias=ebias_t)
        else:
            nc.vector.tensor_scalar(out=e, in0=s2_ps, scalar1=sch_a1,
                                    scalar2=sch_a2, op0=ALU.mult, op1=ALU.add)
        st["etiles"][g] = e
        gpend.append((st, g))
        if g == 2:
            run_tail()
        # rolling two-pair-deferred flush; carries across chunk boundaries so
        # the PE never drains at a chunk seam while the last exps finish
        if len(gpend) > 2:
            fst, fg = gpend.pop(0)
            flush(fst, fg)

    def finish_chunk(st):

        def make_tail(o_ps=st["o_ps"], sums_ps=st["sums_ps"], isl=st["isl"],
                      ic=st["ic"]):
            def tail():
                lns = chunkp.tile([1, CHUNK], BF16, tag="lns", name=f"lns_{ic}")
                nc.scalar.activation(out=lns, in_=sums_ps[0:1, :], func=AF.Ln)
                bc_ps = psum_d.tile([C, CHUNK], F32, tag="d", name=f"bc_{ic}")
                nc.tensor.matmul(bc_ps, lhsT=ones1, rhs=lns, start=True, stop=True)
                recipb = chunkp.tile([C, CHUNK], F32, tag="recipb",
                                     name=f"recipb_{ic}")
                nc.scalar.activation(out=recipb, in_=bc_ps, func=AF.Exp, scale=-1.0)
                o_n = chunkp.tile([C, CHUNK], BF16, tag="o_n", name=f"o_n_{ic}")
                nc.vector.tensor_tensor(out=o_n, in0=o_ps, in1=recipb, op=ALU.mult)
                p_ps = psum_p.tile([C, CHUNK], F32, tag="p", name=f"p_{ic}")
                nc.tensor.matmul(p_ps, lhsT=projT, rhs=o_n, start=True, stop=True)
                out_sb = chunkp.tile([C, CHUNK], F32, tag="out", name=f"out_{ic}")
                nc.vector.scalar_tensor_tensor(out=out_sb, in0=p_ps, scalar=pbe,
                                               in1=x_sb[:, isl],
                                               op0=ALU.add, op1=ALU.add)
                oeng = nc.sync if ic % 2 == 0 else nc.scalar
                oeng.dma_start(out=out_d[:, isl], in_=out_sb)
            return tail

        pending_tail[0] = make_tail()


    st0 = start_chunk(0)
    for s in range(NCHUNK):
        sl = slice(s * CHUNK, (s + 1) * CHUNK)
        nc.scalar.activation(out=h_sb[:, sl], in_=x_sb[:, sl], func=AF.Identity,
                             scale=scale, bias=shift)
        q_ps = psum_s.tile([C, CHUNK], F32, tag="s2", padded_shape=[C, 2 * CHUNK])
        nc.tensor.matmul(q_ps, lhsT=wqT, rhs=h_sb[:, sl], start=True, stop=True)
        nc.vector.tensor_scalar(out=q_sb[:, sl], in0=q_ps, scalar1=bq,
                                scalar2=None, op0=ALU.add)
        k_ps = psum_s.tile([C, CHUNK], F32, tag="s2", padded_shape=[C, 2 * CHUNK])
        nc.tensor.matmul(k_ps, lhsT=wkT, rhs=h_sb[:, sl], start=True, stop=True)
        nc.scalar.copy(out=k_sb[:, sl], in_=k_ps)
        vt_ps = psum_p.tile([C, 4, C], F32, tag="p")
        for ti in range(4):
            nt = 4 * s + ti
            nc.tensor.matmul(vt_ps[:, ti, :], lhsT=h_sb[:, nt * JT:(nt + 1) * JT],
                             rhs=wvT, start=True, stop=True)
        nc.vector.tensor_copy(out=vT[:, 4 * s * C:(4 * s + 4) * C], in_=vt_ps)

    for g in range(NPAIR):
        do_pair(st0, g)
    finish_chunk(st0)

    # ---- P2: attention per i-chunk 1..7 ----
    for ic in range(1, NCHUNK):
        st = start_chunk(ic)
        for g in range(NPAIR):
            do_pair(st, g)
        finish_chunk(st)
    while gpend:
        fst, fg = gpend.pop(0)
        flush(fst, fg)

    run_tail()

    ctx.close()


def _host_consts(gn_w, gn_b, qkv_w, qkv_b, proj_w, proj_b):
    wq = qkv_w[0:C]
    bq = qkv_b[0:C].reshape(C, 1)
    wk = qkv_w[C:2 * C]
    wv = qkv_w[2 * C:3 * C]
    bv = qkv_b[2 * C:3 * C]
    pbe = (proj_b + proj_w @ bv).reshape(C, 1)
    g_ind = np.zeros((C, GROUPS), np.float32)
    g_bc = np.zeros((GROUPS, C), np.float32)
    for p in range(C):
        g_ind[p, p // GSIZE] = 1.0 / GSIZE
        g_bc[p // GSIZE, p] = 1.0
    return {
        "wqT": np.ascontiguousarray(wq.T, np.float32),
        "wkT": np.ascontiguousarray(wk.T, np.float32),
        "wvT": np.ascontiguousarray(wv.T, np.float32),
        "projT": np.ascontiguousarray(proj_w.T, np.float32),
        "bq": bq.astype(np.float32),
        "pbe": pbe.astype(np.float32),
        "gn_w": gn_w.reshape(C, 1).astype(np.float32),
        "gn_b": gn_b.reshape(C, 1).astype(np.float32),
        "g_ind": g_ind,
        "g_bcast": g_bc,
    }


_CACHE = {}


def _hoist_matmul_waits(nc):
    for fn in nc.m.functions:
        for blk in fn.blocks:
            il = blk.instructions
            out = []
            changed = False
            for ins in il:
                si = ins.sync_info
                if (not isinstance(ins, mybir.InstEventSemaphore)
                        and si is not None and len(si.on_wait) > 1):
                    for wi, w in enumerate(si.on_wait[1:]):
                        ev = mybir.InstEventSemaphore(
                            name=f"{ins.name}_hw{wi}", ins=[], outs=[],
                            sync_info=mybir.SyncInfo(on_wait=[w], on_update=[]))
                        ev.engine = ins.engine
                        out.append(ev)
                    ins.sync_info = mybir.SyncInfo(
                        on_wait=[si.on_wait[0]], on_update=si.on_update)
                    changed = True
                out.append(ins)
            if changed:
                il[:] = out


def _build_nc():
    if "nc" in _CACHE:
        return _CACHE["nc"]
    nc = bass.Bass("TRN2", target_bir_lowering=False, debug=False)
    ins = {}
    ins["x"] = nc.declare_dram_parameter("x", [C, N], F32, isOutput=False)[:]
    for nm, shp in [("wqT", [C, C]), ("wkT", [C, C]), ("wvT", [C, C]),
                    ("projT", [C, C]), ("bq", [C, 1]), ("pbe", [C, 1]),
                    ("gn_w", [C, 1]), ("gn_b", [C, 1]),
                    ("g_ind", [C, GROUPS]), ("g_bcast", [GROUPS, C])]:
        ins[nm] = nc.declare_dram_parameter(nm, shp, F32, isOutput=False)[:]
    out = nc.declare_dram_parameter("out", [C, N], F32, isOutput=True)[:]
    with tile.TileContext(nc) as tc:
        attention_block_tile(tc, {"out": out}, ins)
    _hoist_matmul_waits(nc)
    _CACHE["nc"] = nc
    return nc


LAST_EXEC_NS = None
LAST_RESULT = None


def _ensure_ntff_hook():
    import types

    try:
        from antenv import axon_hooks  # noqa: F401
        return
    except ImportError:
        pass
    import antenv
    mod = types.ModuleType("antenv.axon_hooks")
    _hook = [None]
    mod.set_axon_ntff_profile_hook = lambda h: _hook.__setitem__(0, h)
    mod.get_axon_ntff_profile_hook = lambda: _hook[0]
    sys.modules["antenv.axon_hooks"] = mod
    antenv.axon_hooks = mod
    try:
        from trn_agent_boot.trn_boot import _ntff_profile_via_ctypes
        hook = _ntff_profile_via_ctypes("/opt/axon/libaxon_pjrt.so")
        mod.set_axon_ntff_profile_hook(hook)
    except Exception as e:
        print(f"ntff hook unavailable: {e}", file=sys.stderr)


def kernel(x, gn_w, gn_b, qkv_w, qkv_b, proj_w, proj_b):
    global LAST_EXEC_NS, LAST_RESULT
    from concourse.bass_utils import run_bass_kernel_spmd

    x = np.asarray(x, np.float32)
    B = x.shape[0]
    xf = x.reshape(B, C, N)
    consts = _host_consts(np.asarray(gn_w, np.float32), np.asarray(gn_b, np.float32),
                          np.asarray(qkv_w, np.float32), np.asarray(qkv_b, np.float32),
                          np.asarray(proj_w, np.float32), np.asarray(proj_b, np.float32))
    nc = _build_nc()
    in_maps = [dict(consts, x=np.ascontiguousarray(xf[b])) for b in range(NCORES)]
    trace = bool(int(os.environ.get("KERNEL_TRACE", "0")))
    if trace:
        _ensure_ntff_hook()
    res = run_bass_kernel_spmd(nc, in_maps, core_ids=list(range(NCORES)), trace=trace)
    LAST_EXEC_NS = getattr(res, "exec_time_ns", None)
    LAST_RESULT = res
    out = np.stack([res.results[b]["out"] for b in range(NCORES)], axis=0)
    return out.reshape(B, C, 64, 64).astype(np.float32)



# revision 44
# speedup vs baseline: 1.0083x; 1.0083x over previous
"""Fused GroupNorm + self-attention + proj + residual block for TRN2, v3.

Data-parallel over batch (B=8 = 8 cores). Measured-rate-optimal:
- QK in bf16, regular [128]-contract matmuls (1 c/row, same as fp8 on HW).
- PV + denominator in fp8 DoubleRow over paired j-tiles ([128,2,*], 256-deep
  contraction = 2x effective rate). Denominator via ones-matmul on PE.
- exp split across Scalar (native exp -> fp8e5) and Vector (Schraudolph
  int8 bit-trick -> bitcast fp8e5). GPSIMD avoided (software-slow, no PSUM).
"""

import os
import sys
from contextlib import ExitStack

for _p in ("/opt/trn_rl_repo", "/opt/pypackages"):
    if _p not in sys.path:
        sys.path.append(_p)

import numpy as np

import concourse.bass as bass
import concourse.tile as tile
from concourse import mybir

C = 128
N = 4096
GROUPS = 8
GSIZE = C // GROUPS
EPS = 1e-5
NCORES = 8
CHUNK = 512
NCHUNK = N // CHUNK
JT = 128
NJT = N // JT
NPAIR = NJT // 2

F32 = mybir.dt.float32
BF16 = mybir.dt.bfloat16
F8E4 = mybir.dt.float8e4
F8E5 = mybir.dt.float8e5
I8 = mybir.dt.int8
AF = mybir.ActivationFunctionType
ALU = mybir.AluOpType
DR = mybir.MatmulPerfMode.DoubleRow

F_SCALE = C ** -0.5               # score scale, applied inside exp
EBIAS = -2.0                      # exp(s + EBIAS), cancels in normalization
SCH_A = 4.0 / np.log(2.0)         # e5m2 Schraudolph
SCH_B = 60.0 - 4.0 * 0.0430
# exp producer per pair: A=Scalar(ACT), D=Vector(DVE)
EXP_SCHED = "ADAADADAADADAADA"    # 10xA, 6xD


def attention_block_tile(tc, outs, ins):
    nc = tc.nc
    x_d = ins["x"]
    wqT_d = ins["wqT"]
    wkT_d = ins["wkT"]
    wvT_d = ins["wvT"]
    projT_d = ins["projT"]
    bq_d = ins["bq"]
    pbe_d = ins["pbe"]
    gnw_d = ins["gn_w"]
    gnb_d = ins["gn_b"]
    gind_d = ins["g_ind"]
    gbc_d = ins["g_bcast"]
    out_d = outs["out"]

    ctx = ExitStack()
    const = ctx.enter_context(tc.tile_pool(name="const", bufs=1))
    big = ctx.enter_context(tc.tile_pool(name="big", bufs=1))
    small = ctx.enter_context(tc.tile_pool(name="small", bufs=2))
    etile = ctx.enter_context(tc.tile_pool(name="etile", bufs=6))
    chunkp = ctx.enter_context(tc.tile_pool(name="chunkp", bufs=4))
    psum_s = ctx.enter_context(tc.tile_pool(name="psum_s", bufs=2, space="PSUM"))
    psum_o = ctx.enter_context(tc.tile_pool(name="psum_o", bufs=1, space="PSUM"))
    psum_d = ctx.enter_context(tc.tile_pool(name="psum_d", bufs=2, space="PSUM"))
    psum_p = ctx.enter_context(tc.tile_pool(name="psum_p", bufs=1, space="PSUM"))

    dma = nc.sync

    # ---- input DMA first: x chunks gate GroupNorm stats, so their
    # descriptors go to the head of both HWDGE queues ----
    x_sb = big.tile([C, N], F32, tag="x")
    for s in range(8):
        eng = nc.sync if s % 2 == 0 else nc.scalar
        eng.dma_start(out=x_sb[:, s * 512:(s + 1) * 512],
                      in_=x_d[:, s * 512:(s + 1) * 512])

    # ---- constants ----
    wqT_f = const.tile([C, C], F32, tag="wqT_f")
    wkT_f = const.tile([C, C], F32, tag="wkT_f")
    wvT_f = const.tile([C, C], F32, tag="wvT_f")
    projT_f = const.tile([C, C], F32, tag="projT_f")
    dma.dma_start(out=wqT_f, in_=wqT_d)
    dma.dma_start(out=wkT_f, in_=wkT_d)
    dma.dma_start(out=wvT_f, in_=wvT_d)
    dma.dma_start(out=projT_f, in_=projT_d)
    wqT = const.tile([C, C], BF16, tag="wqT")
    wkT = const.tile([C, C], BF16, tag="wkT")
    wvT = const.tile([C, C], BF16, tag="wvT")
    projT = const.tile([C, C], BF16, tag="projT")
    nc.vector.tensor_copy(out=wqT, in_=wqT_f)
    nc.vector.tensor_copy(out=wkT, in_=wkT_f)
    nc.vector.tensor_copy(out=wvT, in_=wvT_f)
    nc.vector.tensor_copy(out=projT, in_=projT_f)
    bq = const.tile([C, 1], F32, tag="bq")
    pbe = const.tile([C, 1], F32, tag="pbe")
    gnw = const.tile([C, 1], F32, tag="gnw")
    gnb = const.tile([C, 1], F32, tag="gnb")
    dma.dma_start(out=bq, in_=bq_d)
    dma.dma_start(out=pbe, in_=pbe_d)
    dma.dma_start(out=gnw, in_=gnw_d)
    dma.dma_start(out=gnb, in_=gnb_d)
    gind = const.tile([C, GROUPS], F32, tag="gind")
    gbc = const.tile([GROUPS, C], F32, tag="gbc")
    dma.dma_start(out=gind, in_=gind_d)
    dma.dma_start(out=gbc, in_=gbc_d)
    ones8 = const.tile([128, 2, 32], F8E4, tag="ones8")
    nc.vector.memset(ones8, 1.0)
    ones1 = const.tile([1, C], BF16, tag="ones1")
    nc.vector.memset(ones1, 1.0)
    ebias_t = const.tile([C, 1], F32, tag="ebias")
    nc.vector.memset(ebias_t, EBIAS)
    warm = const.tile([1, 1], F32, tag="warm")
    nc.vector.memset(warm, 1.0)
    nc.scalar.activation(out=warm, in_=warm, func=AF.Ln)

    # ---- HAM warmup: keep PE busy while the x DMA + GN stats run, so the
    # PE clock gate is already 8/8 when the real matmul burst starts ----
    wt = const.tile([128, 512], BF16, tag="warm_mm")
    nc.vector.memset(wt, 0.25)
    for wi in range(12):
        wp = psum_s.tile([C, CHUNK], F32, tag="s2", padded_shape=[C, 2 * CHUNK],
                         name=f"warm_{wi}")
        nc.tensor.matmul(wp, lhsT=wt[:, 0:128], rhs=wt, start=True, stop=True)

    # ---- GN stats ----
    stats = small.tile([C, 8, 6], F32, tag="gn_stats")
    for s in range(8):
        nc.vector.bn_stats(out=stats[:, s, :], in_=x_sb[:, s * 512:(s + 1) * 512])
    mv = small.tile([C, 2], F32, tag="gn_mv")
    nc.vector.bn_aggr(out=mv, in_=stats)
    stat2 = small.tile([C, 2], F32, tag="gn_stat2")
    nc.vector.tensor_copy(out=stat2[:, 0:1], in_=mv[:, 0:1])
    # E[x^2] = mean^2 + var in one fused op
    nc.vector.scalar_tensor_tensor(out=stat2[:, 1:2], in0=mv[:, 0:1],
                                   scalar=mv[:, 0:1], in1=mv[:, 1:2],
                                   op0=ALU.mult, op1=ALU.add)
    gstats_ps = psum_p.tile([GROUPS, 2], F32, tag="p")
    nc.tensor.matmul(gstats_ps, lhsT=gind, rhs=stat2, start=True, stop=True)
    gstats = small.tile([GROUPS, 2], F32, tag="gn_gstats")
    nc.vector.tensor_copy(out=gstats, in_=gstats_ps)
    # fused: nvar = mean^2 - E[x^2] = -var, then Ln(-1*nvar + eps)
    nvar = small.tile([GROUPS, 1], F32, tag="gn_nvar")
    nc.vector.scalar_tensor_tensor(out=nvar, in0=gstats[:, 0:1],
                                   scalar=gstats[:, 0:1], in1=gstats[:, 1:2],
                                   op0=ALU.mult, op1=ALU.subtract)
    eps_t = const.tile([GROUPS, 1], F32, tag="eps")
    nc.vector.memset(eps_t, EPS)
    glnv = small.tile([GROUPS, 1], F32, tag="gn_glnv")
    nc.scalar.activation(out=glnv, in_=nvar, func=AF.Ln, scale=-1.0, bias=eps_t)
    nc.scalar.activation(out=gstats[:, 1:2], in_=glnv, func=AF.Exp, scale=-0.5)
    chst_ps = psum_p.tile([C, 2], F32, tag="p")
    nc.tensor.matmul(chst_ps, lhsT=gbc, rhs=gstats, start=True, stop=True)
    chst = small.tile([C, 2], F32, tag="gn_chst")
    nc.vector.tensor_copy(out=chst, in_=chst_ps)
    scale = small.tile([C, 1], F32, tag="gn_scale")
    nc.vector.tensor_mul(out=scale, in0=gnw, in1=chst[:, 1:2])
    sm = small.tile([C, 1], F32, tag="gn_sm")
    nc.vector.tensor_mul(out=sm, in0=chst[:, 0:1], in1=scale)
    shift = small.tile([C, 1], F32, tag="gn_shift")
    nc.vector.tensor_tensor(out=shift, in0=gnb, in1=sm, op=ALU.subtract)

    # ---- P1: h, q, k (bf16), vT (fp8e4) per chunk ----
    h_sb = big.tile([C, N], BF16, tag="h")
    q_sb = big.tile([C, N], BF16, tag="q")
    k_sb = big.tile([C, N], BF16, tag="k")
    vT = big.tile([128, NJT * C], F8E4, tag="vT")
    sch_a1 = SCH_A * F_SCALE
    sch_a2 = SCH_B + EBIAS * SCH_A
    pending_tail = [None]

    def run_tail():
        if pending_tail[0] is not None:
            pending_tail[0]()
            pending_tail[0] = None

    gpend = []

    def start_chunk(ic):
        return {
            "ic": ic,
            "isl": slice(ic * CHUNK, (ic + 1) * CHUNK),
            "o_ps": psum_o.tile([C, CHUNK], F32, tag="o", name=f"o_{ic}"),
            "sums_ps": psum_d.tile([32, CHUNK], F32, tag="d", name=f"sums_{ic}"),
            "etiles": {},
        }

    def flush(st, g):
        e2 = st["etiles"].pop(g).bitcast(F8E5).rearrange("p (t i) -> p t i", t=2)
        nc.tensor.matmul(st["o_ps"], lhsT=vT[:, 2 * g * C:(2 * g + 2) * C]
                         .rearrange("p (t c) -> p t c", t=2),
                         rhs=e2, start=(g == 0), stop=(g == NPAIR - 1),
                         perf_mode=DR)
        nc.tensor.matmul(st["sums_ps"], lhsT=ones8,
                         rhs=e2, start=(g == 0), stop=(g == NPAIR - 1),
                         perf_mode=DR, skip_group_check=True)

    def do_pair(st, g):
        isl = st["isl"]
        s2_ps = psum_s.tile([C, 2 * CHUNK], F32, tag="s2")
        for t in range(2):
            jt = 2 * g + t
            nc.tensor.matmul(s2_ps[:, t * CHUNK:(t + 1) * CHUNK],
                             lhsT=k_sb[:, jt * JT:(jt + 1) * JT],
                             rhs=q_sb[:, isl], start=True, stop=True)
        e = etile.tile([C, 2 * CHUNK], I8, tag="e")
        if EXP_SCHED[g] == "A":
            nc.scalar.activation(out=e.bitcast(F8E5), in_=s2_ps, func=AF.Exp,
                                 scale=F_SCALE, bias=ebias_t)
        else:
            nc.vector.tensor_scalar(out=e, in0=s2_ps, scalar1=sch_a1,
                                    scalar2=sch_a2, op0=ALU.mult, op1=ALU.add)
        st["etiles"][g] = e
        gpend.append((st, g))
        if g == 2:
            run_tail()
        # rolling two-pair-deferred flush; carries across chunk boundaries so
        # the PE never drains at a chunk seam while the last exps finish
        if len(gpend) > 2:
            fst, fg = gpend.pop(0)
            flush(fst, fg)

    def finish_chunk(st):

        def make_tail(o_ps=st["o_ps"], sums_ps=st["sums_ps"], isl=st["isl"],
                      ic=st["ic"]):
            def tail():
                lns = chunkp.tile([1, CHUNK], BF16, tag="lns", name=f"lns_{ic}")
                nc.scalar.activation(out=lns, in_=sums_ps[0:1, :], func=AF.Ln)
                bc_ps = psum_d.tile([C, CHUNK], F32, tag="d", name=f"bc_{ic}")
                nc.tensor.matmul(bc_ps, lhsT=ones1, rhs=lns, start=True, stop=True)
                recipb = chunkp.tile([C, CHUNK], F32, tag="recipb",
                                     name=f"recipb_{ic}")
                nc.scalar.activation(out=recipb, in_=bc_ps, func=AF.Exp, scale=-1.0)
                o_n = chunkp.tile([C, CHUNK], BF16, tag="o_n", name=f"o_n_{ic}")
                nc.vector.tensor_tensor(out=o_n, in0=o_ps, in1=recipb, op=ALU.mult)
                p_ps = psum_p.tile([C, CHUNK], F32, tag="p", name=f"p_{ic}")
                nc.tensor.matmul(p_ps, lhsT=projT, rhs=o_n, start=True, stop=True)
                out_sb = chunkp.tile([C, CHUNK], F32, tag="out", name=f"out_{ic}")
                nc.vector.scalar_tensor_tensor(out=out_sb, in0=p_ps, scalar=pbe,
                                               in1=x_sb[:, isl],
                                               op0=ALU.add, op1=ALU.add)
                oeng = nc.sync if ic % 2 == 0 else nc.scalar
                oeng.dma_start(out=out_d[:, isl], in_=out_sb)
            return tail

        pending_tail[0] = make_tail()


    st0 = start_chunk(0)
    for s in range(NCHUNK):
        sl = slice(s * CHUNK, (s + 1) * CHUNK)
        nc.scalar.activation(out=h_sb[:, sl], in_=x_sb[:, sl], func=AF.Identity,
                             scale=scale, bias=shift)
        q_ps = psum_s.tile([C, CHUNK], F32, tag="s2", padded_shape=[C, 2 * CHUNK])
        nc.tensor.matmul(q_ps, lhsT=wqT, rhs=h_sb[:, sl], start=True, stop=True)
        nc.vector.tensor_scalar(out=q_sb[:, sl], in0=q_ps, scalar1=bq,
                                scalar2=None, op0=ALU.add)
        k_ps = psum_s.tile([C, CHUNK], F32, tag="s2", padded_shape=[C, 2 * CHUNK])
        nc.tensor.matmul(k_ps, lhsT=wkT, rhs=h_sb[:, sl], start=True, stop=True)
        nc.scalar.copy(out=k_sb[:, sl], in_=k_ps)
        vt_ps = psum_p.tile([C, 4, C], F32, tag="p")
        for ti in range(4):
            nt = 4 * s + ti
            nc.tensor.matmul(vt_ps[:, ti, :], lhsT=h_sb[:, nt * JT:(nt + 1) * JT],
                             rhs=wvT, start=True, stop=True)
        nc.vector.tensor_copy(out=vT[:, 4 * s * C:(4 * s + 4) * C], in_=vt_ps)

    for g in range(NPAIR):
        do_pair(st0, g)
    finish_chunk(st0)

    # ---- P2: attention per i-chunk 1..7 ----
    for ic in range(1, NCHUNK):
        st = start_chunk(ic)
        for g in range(NPAIR):
            do_pair(st, g)
        finish_chunk(st)
    while gpend:
        fst, fg = gpend.pop(0)
        flush(fst, fg)

    run_tail()

    ctx.close()


def _host_consts(gn_w, gn_b, qkv_w, qkv_b, proj_w, proj_b):
    wq = qkv_w[0:C]
    bq = qkv_b[0:C].reshape(C, 1)
    wk = qkv_w[C:2 * C]
    wv = qkv_w[2 * C:3 * C]
    bv = qkv_b[2 * C:3 * C]
    pbe = (proj_b + proj_w @ bv).reshape(C, 1)
    g_ind = np.zeros((C, GROUPS), np.float32)
    g_bc = np.zeros((GROUPS, C), np.float32)
    for p in range(C):
        g_ind[p, p // GSIZE] = 1.0 / GSIZE
        g_bc[p // GSIZE, p] = 1.0
    return {
        "wqT": np.ascontiguousarray(wq.T, np.float32),
        "wkT": np.ascontiguousarray(wk.T, np.float32),
        "wvT": np.ascontiguousarray(wv.T, np.float32),
        "projT": np.ascontiguousarray(proj_w.T, np.float32),
        "bq": bq.astype(np.float32),
        "pbe": pbe.astype(np.float32),
        "gn_w": gn_w.reshape(C, 1).astype(np.float32),
        "gn_b": gn_b.reshape(C, 1).astype(np.float32),
        "g_ind": g_ind,
        "g_bcast": g_bc,
    }


_CACHE = {}


def _hoist_matmul_waits(nc):
    for fn in nc.m.functions:
        for blk in fn.blocks:
            il = blk.instructions
            out = []
            changed = False
            for ins in il:
                si = ins.sync_info
                if (not isinstance(ins, mybir.InstEventSemaphore)
                        and si is not None and len(si.on_wait) > 1):
                    for wi, w in enumerate(si.on_wait[1:]):
                        ev = mybir.InstEventSemaphore(
                            name=f"{ins.name}_hw{wi}", ins=[], outs=[],
                            sync_info=mybir.SyncInfo(on_wait=[w], on_update=[]))
                        ev.engine = ins.engine
                        out.append(ev)
                    ins.sync_info = mybir.SyncInfo(
                        on_wait=[si.on_wait[0]], on_update=si.on_update)
                    changed = True
                out.append(ins)
            if changed:
                il[:] = out


def _build_nc():
    if "nc" in _CACHE:
        return _CACHE["nc"]
    nc = bass.Bass("TRN2", target_bir_lowering=False, debug=False)
    ins = {}
    ins["x"] = nc.declare_dram_parameter("x", [C, N], F32, isOutput=False)[:]
    for nm, shp in [("wqT", [C, C]), ("wkT", [C, C]), ("wvT", [C, C]),
                    ("projT", [C, C]), ("bq", [C, 1]), ("pbe", [C, 1]),
                    ("gn_w", [C, 1]), ("gn_b", [C, 1]),
                    ("g_ind", [C, GROUPS]), ("g_bcast", [GROUPS, C])]:
        ins[nm] = nc.declare_dram_parameter(nm, shp, F32, isOutput=False)[:]
    out = nc.declare_dram_parameter("out", [C, N], F32, isOutput=True)[:]
    with tile.TileContext(nc) as tc:
        attention_block_tile(tc, {"out": out}, ins)
    _hoist_matmul_waits(nc)
    _CACHE["nc"] = nc
    return nc


LAST_EXEC_NS = None
LAST_RESULT = None


def _ensure_ntff_hook():
    import types

    try:
        from antenv import axon_hooks  # noqa: F401
        return
    except ImportError:
        pass
    import antenv
    mod = types.ModuleType("antenv.axon_hooks")
    _hook = [None]
    mod.set_axon_ntff_profile_hook = lambda h: _hook.__setitem__(0, h)
    mod.get_axon_ntff_profile_hook = lambda: _hook[0]
    sys.modules["antenv.axon_hooks"] = mod
    antenv.axon_hooks = mod
    try:
        from trn_agent_boot.trn_boot import _ntff_profile_via_ctypes
        hook = _ntff_profile_via_ctypes("/opt/axon/libaxon_pjrt.so")
        mod.set_axon_ntff_profile_hook(hook)
    except Exception as e:
        print(f"ntff hook unavailable: {e}", file=sys.stderr)


def kernel(x, gn_w, gn_b, qkv_w, qkv_b, proj_w, proj_b):
    global LAST_EXEC_NS, LAST_RESULT
    from concourse.bass_utils import run_bass_kernel_spmd

    x = np.asarray(x, np.float32)
    B = x.shape[0]
    xf = x.reshape(B, C, N)
    consts = _host_consts(np.asarray(gn_w, np.float32), np.asarray(gn_b, np.float32),
                          np.asarray(qkv_w, np.float32), np.asarray(qkv_b, np.float32),
                          np.asarray(proj_w, np.float32), np.asarray(proj_b, np.float32))
    nc = _build_nc()
    in_maps = [dict(consts, x=np.ascontiguousarray(xf[b])) for b in range(NCORES)]
    trace = bool(int(os.environ.get("KERNEL_TRACE", "0")))
    if trace:
        _ensure_ntff_hook()
    res = run_bass_kernel_spmd(nc, in_maps, core_ids=list(range(NCORES)), trace=trace)
    LAST_EXEC_NS = getattr(res, "exec_time_ns", None)
    LAST_RESULT = res
    out = np.stack([res.results[b]["out"] for b in range(NCORES)], axis=0)
    return out.reshape(B, C, 64, 64).astype(np.float32)



# revision 45
# speedup vs baseline: 1.1865x; 1.1768x over previous
"""Fused GroupNorm + self-attention + proj + residual block for TRN2, v3.

Data-parallel over batch (B=8 = 8 cores). Measured-rate-optimal:
- QK in bf16, regular [128]-contract matmuls (1 c/row, same as fp8 on HW).
- PV + denominator in fp8 DoubleRow over paired j-tiles ([128,2,*], 256-deep
  contraction = 2x effective rate). Denominator via ones-matmul on PE.
- exp split across Scalar (native exp -> fp8e5) and Vector (Schraudolph
  int8 bit-trick -> bitcast fp8e5). GPSIMD avoided (software-slow, no PSUM).
"""

import os
import sys
from contextlib import ExitStack

for _p in ("/opt/trn_rl_repo", "/opt/pypackages"):
    if _p not in sys.path:
        sys.path.append(_p)

import numpy as np

import concourse.bass as bass
import concourse.tile as tile
from concourse import mybir

C = 128
N = 4096
GROUPS = 8
GSIZE = C // GROUPS
EPS = 1e-5
NCORES = 8
CHUNK = 512
NCHUNK = N // CHUNK
JT = 128
NJT = N // JT
NPAIR = NJT // 2

F32 = mybir.dt.float32
BF16 = mybir.dt.bfloat16
F8E4 = mybir.dt.float8e4
F8E5 = mybir.dt.float8e5
I8 = mybir.dt.int8
AF = mybir.ActivationFunctionType
ALU = mybir.AluOpType
DR = mybir.MatmulPerfMode.DoubleRow

F_SCALE = C ** -0.5               # score scale, applied inside exp
EBIAS = -2.0                      # exp(s + EBIAS), cancels in normalization
SCH_A = 4.0 / np.log(2.0)         # e5m2 Schraudolph
SCH_B = 60.0 - 4.0 * 0.0430
# exp producer per pair: A=Scalar(ACT), D=Vector(DVE)
EXP_SCHED = "ADAADADAADADAADA"    # 10xA, 6xD


def attention_block_tile(tc, outs, ins):
    nc = tc.nc
    x_d = ins["x"]
    wqT_d = ins["wqT"]
    wkT_d = ins["wkT"]
    wvT_d = ins["wvT"]
    projT_d = ins["projT"]
    bq_d = ins["bq"]
    pbe_d = ins["pbe"]
    gnw_d = ins["gn_w"]
    gnb_d = ins["gn_b"]
    gind_d = ins["g_ind"]
    gbc_d = ins["g_bcast"]
    out_d = outs["out"]

    ctx = ExitStack()
    const = ctx.enter_context(tc.tile_pool(name="const", bufs=1))
    big = ctx.enter_context(tc.tile_pool(name="big", bufs=1))
    small = ctx.enter_context(tc.tile_pool(name="small", bufs=2))
    etile = ctx.enter_context(tc.tile_pool(name="etile", bufs=6))
    chunkp = ctx.enter_context(tc.tile_pool(name="chunkp", bufs=4))
    psum_s = ctx.enter_context(tc.tile_pool(name="psum_s", bufs=2, space="PSUM"))
    psum_o = ctx.enter_context(tc.tile_pool(name="psum_o", bufs=1, space="PSUM"))
    psum_d = ctx.enter_context(tc.tile_pool(name="psum_d", bufs=2, space="PSUM"))
    psum_p = ctx.enter_context(tc.tile_pool(name="psum_p", bufs=1, space="PSUM"))

    dma = nc.sync

    # ---- input DMA first: x chunks gate GroupNorm stats, so their
    # descriptors go to the head of both HWDGE queues ----
    x_sb = big.tile([C, N], F32, tag="x")
    for s in range(8):
        eng = nc.sync if s % 2 == 0 else nc.scalar
        eng.dma_start(out=x_sb[:, s * 512:(s + 1) * 512],
                      in_=x_d[:, s * 512:(s + 1) * 512])

    # ---- constants ----
    wqT_f = const.tile([C, C], F32, tag="wqT_f")
    wkT_f = const.tile([C, C], F32, tag="wkT_f")
    wvT_f = const.tile([C, C], F32, tag="wvT_f")
    projT_f = const.tile([C, C], F32, tag="projT_f")
    dma.dma_start(out=wqT_f, in_=wqT_d)
    dma.dma_start(out=wkT_f, in_=wkT_d)
    dma.dma_start(out=wvT_f, in_=wvT_d)
    dma.dma_start(out=projT_f, in_=projT_d)
    wqT = const.tile([C, C], BF16, tag="wqT")
    wkT = const.tile([C, C], BF16, tag="wkT")
    wvT = const.tile([C, C], BF16, tag="wvT")
    projT = const.tile([C, C], BF16, tag="projT")
    nc.vector.tensor_copy(out=wqT, in_=wqT_f)
    nc.vector.tensor_copy(out=wkT, in_=wkT_f)
    nc.vector.tensor_copy(out=wvT, in_=wvT_f)
    nc.vector.tensor_copy(out=projT, in_=projT_f)
    bq = const.tile([C, 1], F32, tag="bq")
    pbe = const.tile([C, 1], F32, tag="pbe")
    gnw = const.tile([C, 1], F32, tag="gnw")
    gnb = const.tile([C, 1], F32, tag="gnb")
    dma.dma_start(out=bq, in_=bq_d)
    dma.dma_start(out=pbe, in_=pbe_d)
    dma.dma_start(out=gnw, in_=gnw_d)
    dma.dma_start(out=gnb, in_=gnb_d)
    gind = const.tile([C, GROUPS], F32, tag="gind")
    gbc = const.tile([GROUPS, C], F32, tag="gbc")
    dma.dma_start(out=gind, in_=gind_d)
    dma.dma_start(out=gbc, in_=gbc_d)
    ones8 = const.tile([128, 2, 32], F8E4, tag="ones8")
    nc.vector.memset(ones8, 1.0)
    ones1 = const.tile([1, C], BF16, tag="ones1")
    nc.vector.memset(ones1, 1.0)
    ebias_t = const.tile([C, 1], F32, tag="ebias")
    nc.vector.memset(ebias_t, EBIAS)
    warm = const.tile([1, 1], F32, tag="warm")
    nc.vector.memset(warm, 1.0)
    nc.scalar.activation(out=warm, in_=warm, func=AF.Ln)

    # ---- HAM warmup: keep PE busy while the x DMA + GN stats run, so the
    # PE clock gate is already 8/8 when the real matmul burst starts ----
    wt = const.tile([128, 512], BF16, tag="warm_mm")
    nc.vector.memset(wt, 0.25)
    for wi in range(12):
        wp = psum_s.tile([C, CHUNK], F32, tag="s2", padded_shape=[C, 2 * CHUNK],
                         name=f"warm_{wi}")
        nc.tensor.matmul(wp, lhsT=wt[:, 0:128], rhs=wt, start=True, stop=True)

    # ---- GN stats ----
    stats = small.tile([C, 8, 6], F32, tag="gn_stats")
    for s in range(8):
        nc.vector.bn_stats(out=stats[:, s, :], in_=x_sb[:, s * 512:(s + 1) * 512])
    mv = small.tile([C, 2], F32, tag="gn_mv")
    nc.vector.bn_aggr(out=mv, in_=stats)
    stat2 = small.tile([C, 2], F32, tag="gn_stat2")
    nc.vector.tensor_copy(out=stat2[:, 0:1], in_=mv[:, 0:1])
    # E[x^2] = mean^2 + var in one fused op
    nc.vector.scalar_tensor_tensor(out=stat2[:, 1:2], in0=mv[:, 0:1],
                                   scalar=mv[:, 0:1], in1=mv[:, 1:2],
                                   op0=ALU.mult, op1=ALU.add)
    gstats_ps = psum_p.tile([GROUPS, 2], F32, tag="p")
    nc.tensor.matmul(gstats_ps, lhsT=gind, rhs=stat2, start=True, stop=True)
    gstats = small.tile([GROUPS, 2], F32, tag="gn_gstats")
    nc.vector.tensor_copy(out=gstats, in_=gstats_ps)
    # fused: nvar = mean^2 - E[x^2] = -var, then Ln(-1*nvar + eps)
    nvar = small.tile([GROUPS, 1], F32, tag="gn_nvar")
    nc.vector.scalar_tensor_tensor(out=nvar, in0=gstats[:, 0:1],
                                   scalar=gstats[:, 0:1], in1=gstats[:, 1:2],
                                   op0=ALU.mult, op1=ALU.subtract)
    eps_t = const.tile([GROUPS, 1], F32, tag="eps")
    nc.vector.memset(eps_t, EPS)
    glnv = small.tile([GROUPS, 1], F32, tag="gn_glnv")
    nc.scalar.activation(out=glnv, in_=nvar, func=AF.Ln, scale=-1.0, bias=eps_t)
    nc.scalar.activation(out=gstats[:, 1:2], in_=glnv, func=AF.Exp, scale=-0.5)
    chst_ps = psum_p.tile([C, 2], F32, tag="p")
    nc.tensor.matmul(chst_ps, lhsT=gbc, rhs=gstats, start=True, stop=True)
    chst = small.tile([C, 2], F32, tag="gn_chst")
    nc.vector.tensor_copy(out=chst, in_=chst_ps)
    scale = small.tile([C, 1], F32, tag="gn_scale")
    nc.vector.tensor_mul(out=scale, in0=gnw, in1=chst[:, 1:2])
    sm = small.tile([C, 1], F32, tag="gn_sm")
    nc.vector.tensor_mul(out=sm, in0=chst[:, 0:1], in1=scale)
    shift = small.tile([C, 1], F32, tag="gn_shift")
    nc.vector.tensor_tensor(out=shift, in0=gnb, in1=sm, op=ALU.subtract)

    # second HAM warmup batch: bridges the PE-idle window during the GN
    # finalize chain (7.9us measured, > the ~3.4us HAM MID window) so P1's
    # matmuls start at 2.4 GHz instead of re-throttled 1.2 GHz
    for wi in range(10):
        wp = psum_s.tile([C, CHUNK], F32, tag="s2", padded_shape=[C, 2 * CHUNK],
                         name=f"warm2_{wi}")
        nc.tensor.matmul(wp, lhsT=wt[:, 0:128], rhs=wt, start=True, stop=True)

    # ---- P1: h, q, k (bf16), vT (fp8e4) per chunk ----
    h_sb = big.tile([C, N], BF16, tag="h")
    q_sb = big.tile([C, N], BF16, tag="q")
    k_sb = big.tile([C, N], BF16, tag="k")
    vT = big.tile([128, NJT * C], F8E4, tag="vT")
    sch_a1 = SCH_A * F_SCALE
    sch_a2 = SCH_B + EBIAS * SCH_A
    pending_tail = [None]

    def run_tail():
        if pending_tail[0] is not None:
            pending_tail[0]()
            pending_tail[0] = None

    gpend = []

    def start_chunk(ic):
        return {
            "ic": ic,
            "isl": slice(ic * CHUNK, (ic + 1) * CHUNK),
            "o_ps": psum_o.tile([C, CHUNK], F32, tag="o", name=f"o_{ic}"),
            "sums_ps": psum_d.tile([32, CHUNK], F32, tag="d", name=f"sums_{ic}"),
            "etiles": {},
        }

    def flush(st, g):
        e2 = st["etiles"].pop(g).bitcast(F8E5).rearrange("p (t i) -> p t i", t=2)
        nc.tensor.matmul(st["o_ps"], lhsT=vT[:, 2 * g * C:(2 * g + 2) * C]
                         .rearrange("p (t c) -> p t c", t=2),
                         rhs=e2, start=(g == 0), stop=(g == NPAIR - 1),
                         perf_mode=DR)
        nc.tensor.matmul(st["sums_ps"], lhsT=ones8,
                         rhs=e2, start=(g == 0), stop=(g == NPAIR - 1),
                         perf_mode=DR, skip_group_check=True)

    def do_pair(st, g):
        isl = st["isl"]
        s2_ps = psum_s.tile([C, 2 * CHUNK], F32, tag="s2")
        for t in range(2):
            jt = 2 * g + t
            nc.tensor.matmul(s2_ps[:, t * CHUNK:(t + 1) * CHUNK],
                             lhsT=k_sb[:, jt * JT:(jt + 1) * JT],
                             rhs=q_sb[:, isl], start=True, stop=True)
        e = etile.tile([C, 2 * CHUNK], I8, tag="e")
        if EXP_SCHED[g] == "A":
            nc.scalar.activation(out=e.bitcast(F8E5), in_=s2_ps, func=AF.Exp,
                                 scale=F_SCALE, bias=ebias_t)
        else:
            nc.vector.tensor_scalar(out=e, in0=s2_ps, scalar1=sch_a1,
                                    scalar2=sch_a2, op0=ALU.mult, op1=ALU.add)
        st["etiles"][g] = e
        gpend.append((st, g))
        if g == 2:
            run_tail()
        # rolling two-pair-deferred flush; carries across chunk boundaries so
        # the PE never drains at a chunk seam while the last exps finish
        if len(gpend) > 2:
            fst, fg = gpend.pop(0)
            flush(fst, fg)

    def finish_chunk(st):

        def make_tail(o_ps=st["o_ps"], sums_ps=st["sums_ps"], isl=st["isl"],
                      ic=st["ic"]):
            def tail():
                lns = chunkp.tile([1, CHUNK], BF16, tag="lns", name=f"lns_{ic}")
                nc.scalar.activation(out=lns, in_=sums_ps[0:1, :], func=AF.Ln)
                bc_ps = psum_d.tile([C, CHUNK], F32, tag="d", name=f"bc_{ic}")
                nc.tensor.matmul(bc_ps, lhsT=ones1, rhs=lns, start=True, stop=True)
                recipb = chunkp.tile([C, CHUNK], F32, tag="recipb",
                                     name=f"recipb_{ic}")
                nc.scalar.activation(out=recipb, in_=bc_ps, func=AF.Exp, scale=-1.0)
                o_n = chunkp.tile([C, CHUNK], BF16, tag="o_n", name=f"o_n_{ic}")
                nc.vector.tensor_tensor(out=o_n, in0=o_ps, in1=recipb, op=ALU.mult)
                p_ps = psum_p.tile([C, CHUNK], F32, tag="p", name=f"p_{ic}")
                nc.tensor.matmul(p_ps, lhsT=projT, rhs=o_n, start=True, stop=True)
                out_sb = chunkp.tile([C, CHUNK], F32, tag="out", name=f"out_{ic}")
                nc.vector.scalar_tensor_tensor(out=out_sb, in0=p_ps, scalar=pbe,
                                               in1=x_sb[:, isl],
                                               op0=ALU.add, op1=ALU.add)
                oeng = nc.sync if ic % 2 == 0 else nc.scalar
                oeng.dma_start(out=out_d[:, isl], in_=out_sb)
            return tail

        pending_tail[0] = make_tail()


    st0 = start_chunk(0)
    for s in range(NCHUNK):
        sl = slice(s * CHUNK, (s + 1) * CHUNK)
        nc.scalar.activation(out=h_sb[:, sl], in_=x_sb[:, sl], func=AF.Identity,
                             scale=scale, bias=shift)
        q_ps = psum_s.tile([C, CHUNK], F32, tag="s2", padded_shape=[C, 2 * CHUNK])
        nc.tensor.matmul(q_ps, lhsT=wqT, rhs=h_sb[:, sl], start=True, stop=True)
        nc.vector.tensor_scalar(out=q_sb[:, sl], in0=q_ps, scalar1=bq,
                                scalar2=None, op0=ALU.add)
        k_ps = psum_s.tile([C, CHUNK], F32, tag="s2", padded_shape=[C, 2 * CHUNK])
        nc.tensor.matmul(k_ps, lhsT=wkT, rhs=h_sb[:, sl], start=True, stop=True)
        nc.scalar.copy(out=k_sb[:, sl], in_=k_ps)
        vt_ps = psum_p.tile([C, 4, C], F32, tag="p")
        for ti in range(4):
            nt = 4 * s + ti
            nc.tensor.matmul(vt_ps[:, ti, :], lhsT=h_sb[:, nt * JT:(nt + 1) * JT],
                             rhs=wvT, start=True, stop=True)
        nc.vector.tensor_copy(out=vT[:, 4 * s * C:(4 * s + 4) * C], in_=vt_ps)

    for g in range(NPAIR):
        do_pair(st0, g)
    finish_chunk(st0)

    # ---- P2: attention per i-chunk 1..7 ----
    for ic in range(1, NCHUNK):
        st = start_chunk(ic)
        for g in range(NPAIR):
            do_pair(st, g)
        finish_chunk(st)
    while gpend:
        fst, fg = gpend.pop(0)
        flush(fst, fg)

    run_tail()

    ctx.close()


def _host_consts(gn_w, gn_b, qkv_w, qkv_b, proj_w, proj_b):
    wq = qkv_w[0:C]
    bq = qkv_b[0:C].reshape(C, 1)
    wk = qkv_w[C:2 * C]
    wv = qkv_w[2 * C:3 * C]
    bv = qkv_b[2 * C:3 * C]
    pbe = (proj_b + proj_w @ bv).reshape(C, 1)
    g_ind = np.zeros((C, GROUPS), np.float32)
    g_bc = np.zeros((GROUPS, C), np.float32)
    for p in range(C):
        g_ind[p, p // GSIZE] = 1.0 / GSIZE
        g_bc[p // GSIZE, p] = 1.0
    return {
        "wqT": np.ascontiguousarray(wq.T, np.float32),
        "wkT": np.ascontiguousarray(wk.T, np.float32),
        "wvT": np.ascontiguousarray(wv.T, np.float32),
        "projT": np.ascontiguousarray(proj_w.T, np.float32),
        "bq": bq.astype(np.float32),
        "pbe": pbe.astype(np.float32),
        "gn_w": gn_w.reshape(C, 1).astype(np.float32),
        "gn_b": gn_b.reshape(C, 1).astype(np.float32),
        "g_ind": g_ind,
        "g_bcast": g_bc,
    }


_CACHE = {}


def _hoist_matmul_waits(nc):
    for fn in nc.m.functions:
        for blk in fn.blocks:
            il = blk.instructions
            out = []
            changed = False
            for ins in il:
                si = ins.sync_info
                if (not isinstance(ins, mybir.InstEventSemaphore)
                        and si is not None and len(si.on_wait) > 1):
                    for wi, w in enumerate(si.on_wait[1:]):
                        ev = mybir.InstEventSemaphore(
                            name=f"{ins.name}_hw{wi}", ins=[], outs=[],
                            sync_info=mybir.SyncInfo(on_wait=[w], on_update=[]))
                        ev.engine = ins.engine
                        out.append(ev)
                    ins.sync_info = mybir.SyncInfo(
                        on_wait=[si.on_wait[0]], on_update=si.on_update)
                    changed = True
                out.append(ins)
            if changed:
                il[:] = out


def _build_nc():
    if "nc" in _CACHE:
        return _CACHE["nc"]
    nc = bass.Bass("TRN2", target_bir_lowering=False, debug=False)
    ins = {}
    ins["x"] = nc.declare_dram_parameter("x", [C, N], F32, isOutput=False)[:]
    for nm, shp in [("wqT", [C, C]), ("wkT", [C, C]), ("wvT", [C, C]),
                    ("projT", [C, C]), ("bq", [C, 1]), ("pbe", [C, 1]),
                    ("gn_w", [C, 1]), ("gn_b", [C, 1]),
                    ("g_ind", [C, GROUPS]), ("g_bcast", [GROUPS, C])]:
        ins[nm] = nc.declare_dram_parameter(nm, shp, F32, isOutput=False)[:]
    out = nc.declare_dram_parameter("out", [C, N], F32, isOutput=True)[:]
    with tile.TileContext(nc) as tc:
        attention_block_tile(tc, {"out": out}, ins)
    _hoist_matmul_waits(nc)
    _CACHE["nc"] = nc
    return nc


LAST_EXEC_NS = None
LAST_RESULT = None


def _ensure_ntff_hook():
    import types

    try:
        from antenv import axon_hooks  # noqa: F401
        return
    except ImportError:
        pass
    import antenv
    mod = types.ModuleType("antenv.axon_hooks")
    _hook = [None]
    mod.set_axon_ntff_profile_hook = lambda h: _hook.__setitem__(0, h)
    mod.get_axon_ntff_profile_hook = lambda: _hook[0]
    sys.modules["antenv.axon_hooks"] = mod
    antenv.axon_hooks = mod
    try:
        from trn_agent_boot.trn_boot import _ntff_profile_via_ctypes
        hook = _ntff_profile_via_ctypes("/opt/axon/libaxon_pjrt.so")
        mod.set_axon_ntff_profile_hook(hook)
    except Exception as e:
        print(f"ntff hook unavailable: {e}", file=sys.stderr)


def kernel(x, gn_w, gn_b, qkv_w, qkv_b, proj_w, proj_b):
    global LAST_EXEC_NS, LAST_RESULT
    from concourse.bass_utils import run_bass_kernel_spmd

    x = np.asarray(x, np.float32)
    B = x.shape[0]
    xf = x.reshape(B, C, N)
    consts = _host_consts(np.asarray(gn_w, np.float32), np.asarray(gn_b, np.float32),
                          np.asarray(qkv_w, np.float32), np.asarray(qkv_b, np.float32),
                          np.asarray(proj_w, np.float32), np.asarray(proj_b, np.float32))
    nc = _build_nc()
    in_maps = [dict(consts, x=np.ascontiguousarray(xf[b])) for b in range(NCORES)]
    trace = bool(int(os.environ.get("KERNEL_TRACE", "0")))
    if trace:
        _ensure_ntff_hook()
    res = run_bass_kernel_spmd(nc, in_maps, core_ids=list(range(NCORES)), trace=trace)
    LAST_EXEC_NS = getattr(res, "exec_time_ns", None)
    LAST_RESULT = res
    out = np.stack([res.results[b]["out"] for b in range(NCORES)], axis=0)
    return out.reshape(B, C, 64, 64).astype(np.float32)

